# revision 24
# baseline (speedup 1.0000x reference)
"""Trainium2 Bass kernel for nn_Neighbor_Mean (gnn message passing).

Math: out[b,s,:] = mean_n( mask[b,s,n] * ((pos[idx] + new_h[idx]) @ Wn^T) )
 with new_h[v] = (0 if v==0 else h[v-1]), idx in [0, 2049).
By linearity:  out[b,s,:] = (sum_n T'[idx_eff[b,s,n]]) @ Wn^T / N
 where T'[v] = pos[v] + new_h[v] (bf16 table in DRAM, zero rows past 2048)
 and idx_eff = mask ? idx : sink, sinks spread over the zero rows.

Sharding: data-parallel over batch, one NeuronCore per batch row (B == 8).

Per-core plan (everything chosen for DMA-descriptor efficiency and to keep
the Pool engine free for the gather ucode, which is the throughput limit):
 - table: 17 chunks of (pos + shifted h) added on DVE straight to bf16,
   DMA'd to DRAM scratch. No transform -- Wn is folded in after the reduce.
 - token order t = 512g + 16n + p  (s = 16g + p, g in [0,128), p in [0,16)):
   * wrapped idx layout idxw[t%16, t//16] = idxw[p, 32g + n] reads
     idx_d[16g+p, :] as 128B-contiguous runs -> 2 clean DMAs (idx, msk).
   * gather (non-transposed, HBM source): token t -> partition t%128 =
     16*(n%8) + p, chunk t//128 = 4g + n//8; 1024-token calls (ucode max).
 - masked tokens redirected to sinks SPREAD over zero rows [2049, 2162]:
   a single sink row would serialize the DMA/HBM path (measured 2.1x).
 - reduce per group g: 4 matmuls ps2[h, 16] += lhsT(G_chunk) @ Sel
   (Sel = tiled I16 sums the 8 n-low groups), ACT-copy ps2 -> bf16,
   matmul out3[16, H] = ps2sb^T @ (Wn^T/N), DMA out straight from PSUM.
"""
import sys

sys.path.insert(0, '/opt/trn_rl_repo')

import numpy as np

import concourse.bacc as bacc
import concourse.bass as bass
import concourse.mybir as mybir
import concourse.tile as tile
from concourse.bass_utils import run_bass_kernel_spmd
from concourse.masks import make_identity

B, N, H = 8, 32, 128
NI = 1024            # idxs per dma_gather call (ucode ceiling)
F32 = mybir.dt.float32
I32 = mybir.dt.int32
I16 = mybir.dt.int16
BF16 = mybir.dt.bfloat16


def build_program(S: int = 2048):
    VPOS = S + 1                      # pos_table rows; sinks start here
    NRANKS = (VPOS + 1 + 127) // 128  # table chunks incl. sink rows
    calls = S * N // NI               # gather calls
    gpc = NI // (16 * N)              # s-groups per call

    nc = bacc.Bacc("TRN2", debug=False, num_swdge_queues=4)
    h_d = nc.dram_tensor("h", [S, H], F32, kind="ExternalInput")
    idx_d = nc.dram_tensor("idx", [S, N], I32, kind="ExternalInput")
    msk_d = nc.dram_tensor("msk", [S, N], I32, kind="ExternalInput")
    pos_d = nc.dram_tensor("pos", [VPOS, H], F32, kind="ExternalInput")
    wn_d = nc.dram_tensor("wn", [H, H], F32, kind="ExternalInput")
    out_d = nc.dram_tensor("out", [S, H], F32, kind="ExternalOutput")

    with tile.TileContext(nc) as tc:
        with (
            tc.tile_pool(name="const", bufs=1) as constp,
            tc.tile_pool(name="stage", bufs=3) as stagep,
            tc.tile_pool(name="idxp", bufs=1) as idxp,
            tc.tile_pool(name="gbig", bufs=4) as gbigp,
            tc.tile_pool(name="outp", bufs=8) as outp,
            tc.tile_pool(name="psum", bufs=2, space="PSUM") as psump,
        ):
            # ---- warmup: a tiny early gather absorbs the one-time
            # ucode library load (~9us) off the critical path -----------
            wseed = stagep.tile([128, H], BF16, tag="wseed", bufs=1)
            nc.gpsimd.memset(wseed[:], 0.0)
            widx = stagep.tile([128, 8], I16, tag="widx", bufs=1)
            nc.gpsimd.memset(widx[:], 0)
            wtbl = constp.tile([128, H], BF16, space="DRAM", name="wtbl")
            nc.scalar.dma_start(wtbl[:], wseed[:])
            wdump = constp.tile([1, 8], BF16, space="DRAM", name="wdump")
            wg = stagep.tile([128, 1, H], BF16, tag="wg", bufs=1)
            nc.gpsimd.dma_gather(
                wg[:], wtbl[:], widx[:], 128, 128, H,
                transpose=False, queue_num=0,
            )
            nc.scalar.dma_start(wdump[:], wg[0:1, 0, 0:8])

            # ---- masked indices, wrapped layout (emitted first: the idx
            # chain is the critical path to the first gather) -----------
            # idxw[p, 32g + n] = idx_eff[16g + p, n]; 128B-contiguous DMA.
            # All 2-read DVE ops (copy_predicated) stay in the prologue --
            # the gather ucode streams indices through the POOL/DVE shared
            # SBUF read port; every gather depends on idxbuf transitively.
            acols = S * N // 16
            idxw32 = idxp.tile([16, acols], I32, tag="idxw32")
            mskw32 = idxp.tile([16, acols], I32, tag="mskw32")
            nc.sync.dma_start(
                idxw32[:].rearrange("p (g n) -> p g n", n=N),
                idx_d[:].rearrange("(g p) n -> p g n", p=16),
            )
            nc.scalar.dma_start(
                mskw32[:].rearrange("p (g n) -> p g n", n=N),
                msk_d[:].rearrange("(g p) n -> p g n", p=16),
            )
            # sink spread over zero table rows [2049, 2162]
            sink8 = idxp.tile([16, 8], I32, tag="sink8")
            nc.gpsimd.iota(sink8[:], pattern=[[14, 8]], base=VPOS,
                           channel_multiplier=1)
            idxe32 = idxp.tile([16, acols], I32, tag="idxe32")
            nc.vector.tensor_copy(
                idxe32[:].rearrange("p (r c) -> p r c", c=8),
                sink8[:].rearrange("p (r c) -> p r c", r=1).to_broadcast(
                    [16, acols // 8, 8]),
            )
            nc.vector.copy_predicated(idxe32[:], mskw32[:], idxw32[:])
            # int32 -> int16 (values < 2^15: take low halves)
            idxbuf = idxp.tile([128, acols], I16, tag="idxbuf")
            lo = idxe32[:].bitcast(I16).rearrange("p (e two) -> p e two", two=2)
            nc.vector.tensor_copy(
                idxbuf[0:16, :].rearrange("p (e one) -> p e one", one=1),
                lo[:, :, 0:1],
            )
            # replicate to all 8 16-partition groups by doubling
            nc.sync.dma_start(idxbuf[16:32, :], idxbuf[0:16, :])
            nc.scalar.dma_start(idxbuf[32:64, :], idxbuf[0:32, :])
            nc.sync.dma_start(idxbuf[64:128, :], idxbuf[0:64, :])

            # ---- T' = pos + new_h -> DRAM (bf16), rows >2048 zero -----
            # h-aligned shift trick: T'[r+1] = h[r] + pos[r+1] for
            # r in [0, S) -- ONE add over the whole table, 3 big DMAs.
            tbl_d = constp.tile([NRANKS * 128, H], BF16, space="DRAM",
                                name="tbl2")
            QF = S // 128
            posshift = stagep.tile([128, QF * H], F32, tag="posshift", bufs=1)
            hnat = stagep.tile([128, QF * H], F32, tag="hnat", bufs=1)
            nc.sync.dma_start(
                posshift[:].rearrange("p (q e) -> p q e", e=H),
                pos_d[1:S + 1].rearrange("(q p) e -> p q e", p=128),
            )
            nc.scalar.dma_start(
                hnat[:].rearrange("p (q e) -> p q e", e=H),
                h_d[:].rearrange("(q p) e -> p q e", p=128),
            )
            tsum = stagep.tile([128, QF * H], BF16, tag="tsum", bufs=1)
            nc.vector.tensor_add(tsum[:], posshift[:], hnat[:])
            nc.sync.dma_start(
                tbl_d[1:S + 1].rearrange("(q p) e -> p q e", p=128),
                tsum[:].rearrange("p (q e) -> p q e", e=H),
            )
            # row 0: pos[0] (new_h[0] = 0); rows [S+1, NRANKS*128): zero
            p0 = stagep.tile([1, H], F32, tag="p0", bufs=1)
            nc.scalar.dma_start(p0[:], pos_d[0:1, :])
            t0b = stagep.tile([1, H], BF16, tag="t0b", bufs=1)
            nc.vector.tensor_copy(t0b[:], p0[:])
            nc.scalar.dma_start(tbl_d[0:1, :], t0b[:])
            zsb = stagep.tile([128, H], BF16, tag="zsb", bufs=1)
            nc.gpsimd.memset(zsb[:], 0.0)
            nc.scalar.dma_start(
                tbl_d[S + 1:NRANKS * 128, :], zsb[0:NRANKS * 128 - S - 1, :])

            # ---- Wn^T * (1/N) in bf16: wnt[h,k] = Wn[k,h]/N -----------
            wn_sb = constp.tile([H, H], F32)
            nc.sync.dma_start(wn_sb[:], wn_d[:])
            ident = constp.tile([128, 128], F32)
            make_identity(nc, ident[:])
            wnt_ps = psump.tile([128, H], F32, tag="wnt_ps", bufs=1)
            nc.tensor.transpose(out=wnt_ps[:], in_=wn_sb[:], identity=ident[:])
            wnt = constp.tile([H, H], BF16)
            nc.vector.tensor_scalar_mul(wnt[:], wnt_ps[:], 1.0 / N)

            # ---- Sel[128,16] = I16 tiled down the partition groups ----
            sel16f = constp.tile([16, 16], F32)
            make_identity(nc, sel16f[:])
            sel = constp.tile([128, 16], BF16)
            nc.vector.tensor_copy(sel[0:16, :], sel16f[:])
            nc.scalar.dma_start(sel[16:32, :], sel[0:16, :])
            nc.sync.dma_start(sel[32:64, :], sel[0:32, :])
            nc.scalar.dma_start(sel[64:128, :], sel[0:64, :])

            # ---- main: gather + reduce + Wn fold ----------------------
            for k in range(calls):
                gbig = gbigp.tile([128, NI // 128, H], BF16, tag="gbig")
                nc.gpsimd.dma_gather(
                    gbig[:],
                    tbl_d[:],
                    idxbuf[:, k * (NI // 16):(k + 1) * (NI // 16)],
                    NI, NI, H,
                    transpose=False,
                    queue_num=k % 4,
                )
                for j in range(gpc):
                    g = gpc * k + j
                    # ps2[h, j'] = sum_tok T'row[h] for tok group j'
                    ps2 = psump.tile([128, 16], F32, tag="ps2", bufs=3)
                    for nh in range(N // 8):
                        nc.tensor.matmul(
                            out=ps2[:],
                            lhsT=gbig[:, (N // 8) * j + nh, :],
                            rhs=sel[:],
                            start=(nh == 0),
                            stop=(nh == N // 8 - 1),
                        )
                    p2s = outp.tile([128, 16], BF16, tag="p2s")
                    nc.scalar.copy(p2s[:], ps2[:])
                    out3 = psump.tile([16, H], F32, tag="out3", bufs=3)
                    nc.tensor.matmul(out=out3[:], lhsT=p2s[:], rhs=wnt[:])
                    osb = outp.tile([16, H], F32, tag="osb")
                    nc.vector.tensor_copy(osb[:], out3[:])
                    nc.sync.dma_start(out_d[16 * g:16 * g + 16, :], osb[:])

    nc.compile()
    return nc


_CACHE: dict[int, object] = {}


def _get_program(S: int):
    if S not in _CACHE:
        _CACHE[S] = build_program(S)
    return _CACHE[S]


def kernel(x, h, g, neighbor_index, neighbor_mask, pos_table, Wn):
    """Full inputs in, full output out. x and g are unused by the math
    (g only provides the zero row shape; x is unused in the reference)."""
    h = np.asarray(h)
    idx = np.asarray(neighbor_index)
    msk = np.asarray(neighbor_mask)
    pos = np.ascontiguousarray(np.asarray(pos_table), dtype=np.float32)
    wn = np.ascontiguousarray(np.asarray(Wn), dtype=np.float32)
    b, s, n = idx.shape
    assert (b, n) == (B, N) and h.shape == (B, s, H)

    nc = _get_program(s)
    in_maps = [
        {
            "h": np.ascontiguousarray(h[c], dtype=np.float32),
            "idx": np.ascontiguousarray(idx[c], dtype=np.int32),
            "msk": np.ascontiguousarray(msk[c], dtype=np.int32),
            "pos": pos,
            "wn": wn,
        }
        for c in range(B)
    ]
    res = run_bass_kernel_spmd(nc, in_maps, core_ids=list(range(B)))
    return np.stack([res.results[c]["out"] for c in range(B)], axis=0)


# revision 25
# speedup vs baseline: 1.0033x; 1.0033x over previous
"""Trainium2 Bass kernel for nn_Neighbor_Mean (gnn message passing).

Math: out[b,s,:] = mean_n( mask[b,s,n] * ((pos[idx] + new_h[idx]) @ Wn^T) )
 with new_h[v] = (0 if v==0 else h[v-1]), idx in [0, 2049).
By linearity:  out[b,s,:] = (sum_n T'[idx_eff[b,s,n]]) @ Wn^T / N
 where T'[v] = pos[v] + new_h[v] (bf16 table in DRAM, zero rows past 2048)
 and idx_eff = mask ? idx : sink, sinks spread over the zero rows.

Sharding: data-parallel over batch, one NeuronCore per batch row (B == 8).

Per-core plan (everything chosen for DMA-descriptor efficiency and to keep
the Pool engine free for the gather ucode, which is the throughput limit):
 - table: 17 chunks of (pos + shifted h) added on DVE straight to bf16,
   DMA'd to DRAM scratch. No transform -- Wn is folded in after the reduce.
 - token order t = 512g + 16n + p  (s = 16g + p, g in [0,128), p in [0,16)):
   * wrapped idx layout idxw[t%16, t//16] = idxw[p, 32g + n] reads
     idx_d[16g+p, :] as 128B-contiguous runs -> 2 clean DMAs (idx, msk).
   * gather (non-transposed, HBM source): token t -> partition t%128 =
     16*(n%8) + p, chunk t//128 = 4g + n//8; 1024-token calls (ucode max).
 - masked tokens redirected to sinks SPREAD over zero rows [2049, 2162]:
   a single sink row would serialize the DMA/HBM path (measured 2.1x).
 - reduce per group g: 4 matmuls ps2[h, 16] += lhsT(G_chunk) @ Sel
   (Sel = tiled I16 sums the 8 n-low groups), ACT-copy ps2 -> bf16,
   matmul out3[16, H] = ps2sb^T @ (Wn^T/N), DMA out straight from PSUM.
"""
import sys

sys.path.insert(0, '/opt/trn_rl_repo')

import numpy as np

import concourse.bacc as bacc
import concourse.bass as bass
import concourse.mybir as mybir
import concourse.tile as tile
from concourse.bass_utils import run_bass_kernel_spmd
from concourse.masks import make_identity

B, N, H = 8, 32, 128
NI = 1024            # idxs per dma_gather call (ucode ceiling)
F32 = mybir.dt.float32
I32 = mybir.dt.int32
I16 = mybir.dt.int16
BF16 = mybir.dt.bfloat16


def build_program(S: int = 2048):
    VPOS = S + 1                      # pos_table rows; sinks start here
    NRANKS = (VPOS + 1 + 127) // 128  # table chunks incl. sink rows
    calls = S * N // NI               # gather calls
    gpc = NI // (16 * N)              # s-groups per call

    nc = bacc.Bacc("TRN2", debug=False, num_swdge_queues=4)
    h_d = nc.dram_tensor("h", [S, H], F32, kind="ExternalInput")
    idx_d = nc.dram_tensor("idx", [S, N], I32, kind="ExternalInput")
    msk_d = nc.dram_tensor("msk", [S, N], I32, kind="ExternalInput")
    pos_d = nc.dram_tensor("pos", [VPOS, H], F32, kind="ExternalInput")
    wn_d = nc.dram_tensor("wn", [H, H], F32, kind="ExternalInput")
    out_d = nc.dram_tensor("out", [S, H], F32, kind="ExternalOutput")

    with tile.TileContext(nc) as tc:
        with (
            tc.tile_pool(name="const", bufs=1) as constp,
            tc.tile_pool(name="stage", bufs=3) as stagep,
            tc.tile_pool(name="idxp", bufs=1) as idxp,
            tc.tile_pool(name="gbig", bufs=4) as gbigp,
            tc.tile_pool(name="outp", bufs=8) as outp,
            tc.tile_pool(name="psum", bufs=2, space="PSUM") as psump,
        ):
            # ---- warmup: a tiny early gather absorbs the one-time
            # ucode library load (~9us) off the critical path -----------
            wseed = stagep.tile([128, H], BF16, tag="wseed", bufs=1)
            nc.gpsimd.memset(wseed[:], 0.0)
            widx = stagep.tile([128, 8], I16, tag="widx", bufs=1)
            nc.gpsimd.memset(widx[:], 0)
            wtbl = constp.tile([128, H], BF16, space="DRAM", name="wtbl")
            nc.scalar.dma_start(wtbl[:], wseed[:])
            wdump = constp.tile([4, 8], BF16, space="DRAM", name="wdump")
            for wq in range(4):
                wg = stagep.tile([128, 1, H], BF16, tag="wg", bufs=4)
                nc.gpsimd.dma_gather(
                    wg[:], wtbl[:], widx[:], 128, 128, H,
                    transpose=False, queue_num=wq,
                )
                nc.scalar.dma_start(wdump[wq:wq + 1, :], wg[0:1, 0, 0:8])

            # ---- masked indices, wrapped layout (emitted first: the idx
            # chain is the critical path to the first gather) -----------
            # idxw[p, 32g + n] = idx_eff[16g + p, n]; 128B-contiguous DMA.
            # All 2-read DVE ops (copy_predicated) stay in the prologue --
            # the gather ucode streams indices through the POOL/DVE shared
            # SBUF read port; every gather depends on idxbuf transitively.
            acols = S * N // 16
            idxw32 = idxp.tile([16, acols], I32, tag="idxw32")
            mskw32 = idxp.tile([16, acols], I32, tag="mskw32")
            nc.sync.dma_start(
                idxw32[:].rearrange("p (g n) -> p g n", n=N),
                idx_d[:].rearrange("(g p) n -> p g n", p=16),
            )
            nc.scalar.dma_start(
                mskw32[:].rearrange("p (g n) -> p g n", n=N),
                msk_d[:].rearrange("(g p) n -> p g n", p=16),
            )
            # sink spread over zero table rows [2049, 2162]
            sink8 = idxp.tile([16, 8], I32, tag="sink8")
            nc.gpsimd.iota(sink8[:], pattern=[[14, 8]], base=VPOS,
                           channel_multiplier=1)
            idxe32 = idxp.tile([16, acols], I32, tag="idxe32")
            nc.vector.tensor_copy(
                idxe32[:].rearrange("p (r c) -> p r c", c=8),
                sink8[:].rearrange("p (r c) -> p r c", r=1).to_broadcast(
                    [16, acols // 8, 8]),
            )
            nc.vector.copy_predicated(idxe32[:], mskw32[:], idxw32[:])
            # int32 -> int16 (values < 2^15: take low halves)
            idxbuf = idxp.tile([128, acols], I16, tag="idxbuf")
            lo = idxe32[:].bitcast(I16).rearrange("p (e two) -> p e two", two=2)
            nc.vector.tensor_copy(
                idxbuf[0:16, :].rearrange("p (e one) -> p e one", one=1),
                lo[:, :, 0:1],
            )
            # replicate to all 8 16-partition groups by doubling
            nc.sync.dma_start(idxbuf[16:32, :], idxbuf[0:16, :])
            nc.scalar.dma_start(idxbuf[32:64, :], idxbuf[0:32, :])
            nc.sync.dma_start(idxbuf[64:128, :], idxbuf[0:64, :])

            # ---- T' = pos + new_h -> DRAM (bf16), rows >2048 zero -----
            # h-aligned shift trick: T'[r+1] = h[r] + pos[r+1] for
            # r in [0, S) -- ONE add over the whole table, 3 big DMAs.
            tbl_d = constp.tile([NRANKS * 128, H], BF16, space="DRAM",
                                name="tbl2")
            QF = S // 128
            posshift = stagep.tile([128, QF * H], F32, tag="posshift", bufs=1)
            hnat = stagep.tile([128, QF * H], F32, tag="hnat", bufs=1)
            nc.sync.dma_start(
                posshift[:].rearrange("p (q e) -> p q e", e=H),
                pos_d[1:S + 1].rearrange("(q p) e -> p q e", p=128),
            )
            nc.scalar.dma_start(
                hnat[:].rearrange("p (q e) -> p q e", e=H),
                h_d[:].rearrange("(q p) e -> p q e", p=128),
            )
            tsum = stagep.tile([128, QF * H], BF16, tag="tsum", bufs=1)
            nc.vector.tensor_add(tsum[:], posshift[:], hnat[:])
            nc.sync.dma_start(
                tbl_d[1:S + 1].rearrange("(q p) e -> p q e", p=128),
                tsum[:].rearrange("p (q e) -> p q e", e=H),
            )
            # row 0: pos[0] (new_h[0] = 0); rows [S+1, NRANKS*128): zero
            p0 = stagep.tile([1, H], F32, tag="p0", bufs=1)
            nc.scalar.dma_start(p0[:], pos_d[0:1, :])
            t0b = stagep.tile([1, H], BF16, tag="t0b", bufs=1)
            nc.vector.tensor_copy(t0b[:], p0[:])
            nc.scalar.dma_start(tbl_d[0:1, :], t0b[:])
            zsb = stagep.tile([128, H], BF16, tag="zsb", bufs=1)
            nc.gpsimd.memset(zsb[:], 0.0)
            nc.scalar.dma_start(
                tbl_d[S + 1:NRANKS * 128, :], zsb[0:NRANKS * 128 - S - 1, :])

            # ---- Wn^T * (1/N) in bf16: wnt[h,k] = Wn[k,h]/N -----------
            wn_sb = constp.tile([H, H], F32)
            nc.sync.dma_start(wn_sb[:], wn_d[:])
            ident = constp.tile([128, 128], F32)
            make_identity(nc, ident[:])
            wnt_ps = psump.tile([128, H], F32, tag="wnt_ps", bufs=1)
            nc.tensor.transpose(out=wnt_ps[:], in_=wn_sb[:], identity=ident[:])
            wnt = constp.tile([H, H], BF16)
            nc.vector.tensor_scalar_mul(wnt[:], wnt_ps[:], 1.0 / N)

            # ---- Sel[128,16] = I16 tiled down the partition groups ----
            sel16f = constp.tile([16, 16], F32)
            make_identity(nc, sel16f[:])
            sel = constp.tile([128, 16], BF16)
            nc.vector.tensor_copy(sel[0:16, :], sel16f[:])
            nc.scalar.dma_start(sel[16:32, :], sel[0:16, :])
            nc.sync.dma_start(sel[32:64, :], sel[0:32, :])
            nc.scalar.dma_start(sel[64:128, :], sel[0:64, :])

            # ---- main: gather + reduce + Wn fold ----------------------
            for k in range(calls):
                gbig = gbigp.tile([128, NI // 128, H], BF16, tag="gbig")
                nc.gpsimd.dma_gather(
                    gbig[:],
                    tbl_d[:],
                    idxbuf[:, k * (NI // 16):(k + 1) * (NI // 16)],
                    NI, NI, H,
                    transpose=False,
                    queue_num=k % 4,
                )
                for j in range(gpc):
                    g = gpc * k + j
                    # ps2[h, j'] = sum_tok T'row[h] for tok group j'
                    ps2 = psump.tile([128, 16], F32, tag="ps2", bufs=3)
                    for nh in range(N // 8):
                        nc.tensor.matmul(
                            out=ps2[:],
                            lhsT=gbig[:, (N // 8) * j + nh, :],
                            rhs=sel[:],
                            start=(nh == 0),
                            stop=(nh == N // 8 - 1),
                        )
                    p2s = outp.tile([128, 16], BF16, tag="p2s")
                    nc.scalar.copy(p2s[:], ps2[:])
                    out3 = psump.tile([16, H], F32, tag="out3", bufs=3)
                    nc.tensor.matmul(out=out3[:], lhsT=p2s[:], rhs=wnt[:])
                    osb = outp.tile([16, H], F32, tag="osb")
                    nc.vector.tensor_copy(osb[:], out3[:])
                    nc.sync.dma_start(out_d[16 * g:16 * g + 16, :], osb[:])

    nc.compile()
    return nc


_CACHE: dict[int, object] = {}


def _get_program(S: int):
    if S not in _CACHE:
        _CACHE[S] = build_program(S)
    return _CACHE[S]


def kernel(x, h, g, neighbor_index, neighbor_mask, pos_table, Wn):
    """Full inputs in, full output out. x and g are unused by the math
    (g only provides the zero row shape; x is unused in the reference)."""
    h = np.asarray(h)
    idx = np.asarray(neighbor_index)
    msk = np.asarray(neighbor_mask)
    pos = np.ascontiguousarray(np.asarray(pos_table), dtype=np.float32)
    wn = np.ascontiguousarray(np.asarray(Wn), dtype=np.float32)
    b, s, n = idx.shape
    assert (b, n) == (B, N) and h.shape == (B, s, H)

    nc = _get_program(s)
    in_maps = [
        {
            "h": np.ascontiguousarray(h[c], dtype=np.float32),
            "idx": np.ascontiguousarray(idx[c], dtype=np.int32),
            "msk": np.ascontiguousarray(msk[c], dtype=np.int32),
            "pos": pos,
            "wn": wn,
        }
        for c in range(B)
    ]
    res = run_bass_kernel_spmd(nc, in_maps, core_ids=list(range(B)))
    return np.stack([res.results[c]["out"] for c in range(B)], axis=0)
